# revision 1
# baseline (speedup 1.0000x reference)
"""Trainium2 Bass kernel v4 for the FM + MLP embedding-lookup model.

Gather contract (established by HW probing): one offset per dest partition
per indirect DMA -> 26 per-field DMAs per 128-row chunk (416 per core).
Optimizations vs the v1 baseline:
  1. PAIRED table: host builds w2p[r] = [w2[r] | w2[r]^2] as f16 (256B rows).
     Each descriptor carries the square for free - no on-device squares, and
     the fold tree + PE transpose-reduce handle s and ss simultaneously.
  2. 4 SWDGE queues, round-robin per DMA, to parallelize Q7 descriptor gen.
  3. Field reduction: DVE pairwise folds (26->13->6+carry, packed f16 2x)
     then 7 PE transpose-matmuls accumulating [sT; ssT] into one PSUM tile.
  4. h1 bias folded into the matmul contraction (ones-row in fmTe);
     per-chunk outputs accumulated into one PSUM tile via column-masked
     wout/ones stationaries (b_out + bias folded in); single output DMA.

Sharding: data-parallel over batch across 8 cores, paired table replicated.
"""
import sys

for _p in ("/opt/trn_rl_repo", "/opt/pypackages"):
    if _p not in sys.path:
        sys.path.append(_p)

import numpy as np

import concourse.bacc as bacc
import concourse.mybir as mybir
import concourse.tile as tile
from concourse.bass import IndirectOffsetOnAxis

B, F, V, D = 16384, 26, 100000, 64
H1, H2 = 256, 128
E = 2 * D                 # paired row: [e | e^2], 128 f16 elems = 256B
NCORES = 8
BC = B // NCORES          # 2048 rows per core
P = 128                   # chunk rows == SBUF partitions
NCHUNK = BC // P          # 16
NQ = 4                    # SWDGE queues

f32 = mybir.dt.float32
f16 = mybir.dt.float16
i32 = mybir.dt.int32

_cache: dict = {}


def _indirect_gather_q(nc, out, in_, offset_ap, queue_num):
    """indirect_dma_start pinned to qPoolDynamic{queue_num}."""
    inst = nc.gpsimd.indirect_dma_start(
        out=out, out_offset=None, in_=in_,
        in_offset=IndirectOffsetOnAxis(ap=offset_ap, axis=0),
    )
    if queue_num:
        inst.ins.queue = f"qPoolDynamic{queue_num}"
    return inst


def build_program(repeats: int = 1, nq: int = NQ):
    nc = bacc.Bacc(None, target_bir_lowering=False, num_swdge_queues=nq)

    idx_d = nc.dram_tensor("idx", [P, NCHUNK * F], i32, kind="ExternalInput")
    w2_d = nc.dram_tensor("w2p", [F * V, E], f16, kind="ExternalInput")
    wh1_d = nc.dram_tensor("wh1e", [D + 1, H1], f32, kind="ExternalInput")
    wh2_d = nc.dram_tensor("wh2", [H1, H2], f32, kind="ExternalInput")
    bh2_d = nc.dram_tensor("bh2", [H2, 1], f32, kind="ExternalInput")
    wout_d = nc.dram_tensor("woutm", [H2, NCHUNK * NCHUNK], f32, kind="ExternalInput")
    ones_d = nc.dram_tensor("onesm", [D + 1, NCHUNK * NCHUNK], f32, kind="ExternalInput")
    iden_d = nc.dram_tensor("iden", [P, P], f16, kind="ExternalInput")
    out_d = nc.dram_tensor("out", [NCHUNK, P], f32, kind="ExternalOutput")

    with tile.TileContext(nc) as tc:
        with tc.tile_pool(name="const", bufs=1) as cp, \
             tc.tile_pool(name="emb", bufs=8) as ep, \
             tc.tile_pool(name="fold", bufs=4) as fp_, \
             tc.tile_pool(name="small", bufs=3) as sp, \
             tc.tile_pool(name="psumS", bufs=2, space="PSUM") as pps, \
             tc.tile_pool(name="psumM", bufs=1, space="PSUM") as ppm, \
             tc.tile_pool(name="psumF", bufs=1, space="PSUM") as ppf:
            idx_t = cp.tile([P, NCHUNK * F], i32)
            wh1_t = cp.tile([D + 1, H1], f32)
            wh2a_t = cp.tile([P, H2], f32)
            wh2b_t = cp.tile([P, H2], f32)
            bh2_t = cp.tile([H2, 1], f32)
            wout_t = cp.tile([H2, NCHUNK * NCHUNK], f32)
            ones_t = cp.tile([D + 1, NCHUNK * NCHUNK], f32)
            iden_t = cp.tile([P, P], f16)
            nc.sync.dma_start(idx_t[:], idx_d[:])
            nc.sync.dma_start(wh1_t[:], wh1_d[:])
            nc.sync.dma_start(wh2a_t[:], wh2_d[0:P, :])
            nc.sync.dma_start(wh2b_t[:], wh2_d[P : 2 * P, :])
            nc.sync.dma_start(bh2_t[:], bh2_d[:])
            nc.sync.dma_start(wout_t[:], wout_d[:])
            nc.sync.dma_start(ones_t[:], ones_d[:])
            nc.sync.dma_start(iden_t[:], iden_d[:])

            qn = 0
            for _r in range(repeats):
                fin_p = ppf.tile([NCHUNK, P], f32, tag="fin16", space="PSUM")
                for c in range(NCHUNK):
                    emb = ep.tile([P, F * E], f16, tag="emb")
                    for f in range(F):
                        _indirect_gather_q(
                            nc, emb[:, f * E : (f + 1) * E], w2_d[:],
                            idx_t[:, c * F + f : c * F + f + 1], qn % nq,
                        )
                        qn += 1

                    # Pairwise folds (s and ss together): 26 -> 13 -> 6 + carry
                    e13 = fp_.tile([P, 13 * E], f16, tag="e13")
                    nc.vector.tensor_tensor(
                        out=e13[:], in0=emb[:, : 13 * E], in1=emb[:, 13 * E :],
                        op=mybir.AluOpType.add,
                    )
                    e6 = fp_.tile([P, 6 * E], f16, tag="e6")
                    nc.vector.tensor_tensor(
                        out=e6[:], in0=e13[:, : 6 * E], in1=e13[:, 6 * E : 12 * E],
                        op=mybir.AluOpType.add,
                    )

                    # 7 transpose-matmuls accumulate [sT; ssT] = [128, P] PSUM
                    # (partitions 0:64 = sT, 64:128 = ssT).
                    sT_p = pps.tile([E, P], f32, tag="sT", space="PSUM")
                    for k in range(6):
                        nc.tensor.matmul(
                            out=sT_p[:], lhsT=e6[:, k * E : (k + 1) * E],
                            rhs=iden_t[:], start=(k == 0), stop=False,
                        )
                    nc.tensor.matmul(
                        out=sT_p[:], lhsT=e13[:, 12 * E : 13 * E], rhs=iden_t[:],
                        start=False, stop=True,
                    )

                    # fmTe [D+1, P]: rows 0:D = sT^2 - ssT, row D = 1.0.
                    fmTe = sp.tile([D + 1, P], f32, tag="fmTe")
                    nc.scalar.square(fmTe[0:D, :], sT_p[0:D, :])
                    nc.vector.memset(fmTe[D : D + 1, :], 1.0)
                    nc.vector.tensor_tensor(
                        out=fmTe[0:D, :], in0=fmTe[0:D, :], in1=sT_p[D : 2 * D, :],
                        op=mybir.AluOpType.subtract,
                    )

                    h1_p = ppm.tile([P, H1], f32, tag="h1", space="PSUM")
                    nc.tensor.matmul(
                        out=h1_p[:, 0:P], lhsT=wh1_t[:, 0:P], rhs=fmTe[:],
                        start=True, stop=True,
                    )
                    nc.tensor.matmul(
                        out=h1_p[:, P : 2 * P], lhsT=wh1_t[:, P : 2 * P],
                        rhs=fmTe[:], start=True, stop=True, skip_group_check=True,
                    )
                    h1 = sp.tile([P, H1], f32, tag="h1_sb")
                    nc.scalar.activation(
                        h1[:], h1_p[:], mybir.ActivationFunctionType.Sigmoid,
                        scale=0.005,
                    )

                    h2_p = ppm.tile([P, P], f32, tag="h2", space="PSUM")
                    nc.tensor.matmul(
                        out=h2_p[:], lhsT=wh2a_t[:], rhs=h1[:, 0:P],
                        start=True, stop=False,
                    )
                    nc.tensor.matmul(
                        out=h2_p[:], lhsT=wh2b_t[:], rhs=h1[:, P : 2 * P],
                        start=False, stop=True,
                    )
                    h2 = sp.tile([P, P], f32, tag="h2_sb")
                    nc.scalar.activation(
                        h2[:], h2_p[:], mybir.ActivationFunctionType.Sigmoid,
                        bias=bh2_t[:, 0:1],
                    )

                    # Chunk c deposits its output row into partition c of the
                    # shared fin16 PSUM tile via column-masked stationaries.
                    nc.tensor.matmul(
                        out=fin_p[:], lhsT=wout_t[:, c * NCHUNK : (c + 1) * NCHUNK],
                        rhs=h2[:], start=(c == 0), stop=False,
                    )
                    nc.tensor.matmul(
                        out=fin_p[:], lhsT=ones_t[:, c * NCHUNK : (c + 1) * NCHUNK],
                        rhs=fmTe[:], start=False, stop=(c == NCHUNK - 1),
                    )
                orow = sp.tile([NCHUNK, P], f32, tag="orow16")
                nc.vector.tensor_scalar_add(orow[:], fin_p[:], 0.0)
                nc.sync.dma_start(out_d[:], orow[:])
    nc.compile()
    return nc


def prep_inputs(cat_feat, W2, W_h1, b_h1, W_h2, b_h2, W_out, b_out, bias):
    cat = np.asarray(cat_feat).astype(np.int64)
    flat = (np.arange(F, dtype=np.int64)[None, :] * V + cat).astype(np.int32)
    idx = flat.reshape(NCORES, NCHUNK, P, F).transpose(0, 2, 1, 3).reshape(
        NCORES, P, NCHUNK * F
    )
    w2f = np.asarray(W2, dtype=np.float32).reshape(F * V, D)
    w2p = np.empty((F * V, E), dtype=np.float16)
    w2p[:, :D] = w2f.astype(np.float16)
    w2p[:, D:] = np.square(w2f).astype(np.float16)
    wh1e = np.concatenate(
        [
            np.asarray(W_h1, dtype=np.float32),
            (np.asarray(b_h1, dtype=np.float32) / np.float32(0.005))[None, :],
        ],
        axis=0,
    )
    wh2 = np.ascontiguousarray(np.asarray(W_h2, dtype=np.float32))
    bh2 = np.asarray(b_h2, dtype=np.float32).reshape(H2, 1)
    cb = np.float32(np.asarray(b_out).reshape(-1)[0]) + np.float32(
        np.asarray(bias).reshape(-1)[0]
    )
    # Column-masked stationaries: chunk c uses cols [16c,16c+16); col 16c+c
    # holds the real vector (ones-row carries cb), others zero.
    woutm = np.zeros((H2, NCHUNK * NCHUNK), dtype=np.float32)
    onesm = np.zeros((D + 1, NCHUNK * NCHUNK), dtype=np.float32)
    for c in range(NCHUNK):
        woutm[:, c * NCHUNK + c] = np.asarray(W_out, dtype=np.float32).reshape(-1)
        onesm[:D, c * NCHUNK + c] = 0.005
        onesm[D, c * NCHUNK + c] = cb
    iden = np.eye(P, dtype=np.float16)
    common = {
        "w2p": w2p, "wh1e": np.ascontiguousarray(wh1e), "wh2": wh2, "bh2": bh2,
        "woutm": woutm, "onesm": onesm, "iden": iden,
    }
    return [dict(common, idx=np.ascontiguousarray(idx[c])) for c in range(NCORES)]


def kernel(**inputs) -> np.ndarray:
    from concourse.bass_utils import run_bass_kernel_spmd

    if "nc" not in _cache:
        _cache["nc"] = build_program()
    nc = _cache["nc"]
    in_maps = prep_inputs(**inputs)
    res = run_bass_kernel_spmd(nc, in_maps, list(range(NCORES)))
    out = np.concatenate(
        [np.asarray(res.results[c]["out"]).reshape(BC) for c in range(NCORES)]
    )
    return out.astype(np.float32)[:, None]



# revision 3
# speedup vs baseline: 1.0283x; 1.0283x over previous
"""Trainium2 Bass kernel v4 for the FM + MLP embedding-lookup model.

Gather contract (established by HW probing): one offset per dest partition
per indirect DMA -> 26 per-field DMAs per 128-row chunk (416 per core).
Optimizations vs the v1 baseline:
  1. PAIRED table: host builds w2p[r] = [w2[r] | w2[r]^2] as f16 (256B rows).
     Each descriptor carries the square for free - no on-device squares, and
     the fold tree + PE transpose-reduce handle s and ss simultaneously.
  2. 4 SWDGE queues, round-robin per DMA, to parallelize Q7 descriptor gen.
     v5: emb pool deepened to 16 bufs (whole batch resident) so the Pool
     engine's gather stream never head-of-line blocks on fold/PE consumers;
     measured pure-stream rate is ~0.8us/gather vs ~1.4us effective in v4.
  3. Field reduction: DVE pairwise folds (26->13->6+carry, packed f16 2x)
     then 7 PE transpose-matmuls accumulating [sT; ssT] into one PSUM tile.
  4. h1 bias folded into the matmul contraction (ones-row in fmTe);
     per-chunk outputs accumulated into one PSUM tile via column-masked
     wout/ones stationaries (b_out + bias folded in); single output DMA.

Sharding: data-parallel over batch across 8 cores, paired table replicated.
"""
import sys

for _p in ("/opt/trn_rl_repo", "/opt/pypackages"):
    if _p not in sys.path:
        sys.path.append(_p)

import numpy as np

import concourse.bacc as bacc
import concourse.mybir as mybir
import concourse.tile as tile
from concourse.bass import IndirectOffsetOnAxis

B, F, V, D = 16384, 26, 100000, 64
H1, H2 = 256, 128
E = 2 * D                 # paired row: [e | e^2], 128 f16 elems = 256B
NCORES = 8
BC = B // NCORES          # 2048 rows per core
P = 128                   # chunk rows == SBUF partitions
NCHUNK = BC // P          # 16
NQ = 4                    # SWDGE queues

f32 = mybir.dt.float32
f16 = mybir.dt.float16
i32 = mybir.dt.int32

_cache: dict = {}


def _indirect_gather_q(nc, out, in_, offset_ap, queue_num):
    """indirect_dma_start pinned to qPoolDynamic{queue_num}."""
    inst = nc.gpsimd.indirect_dma_start(
        out=out, out_offset=None, in_=in_,
        in_offset=IndirectOffsetOnAxis(ap=offset_ap, axis=0),
    )
    if queue_num:
        inst.ins.queue = f"qPoolDynamic{queue_num}"
    return inst


def build_program(repeats: int = 1, nq: int = NQ):
    nc = bacc.Bacc(None, target_bir_lowering=False, num_swdge_queues=nq)

    idx_d = nc.dram_tensor("idx", [P, NCHUNK * F], i32, kind="ExternalInput")
    w2_d = nc.dram_tensor("w2p", [F * V, E], f16, kind="ExternalInput")
    wh1_d = nc.dram_tensor("wh1e", [D + 1, H1], f32, kind="ExternalInput")
    wh2_d = nc.dram_tensor("wh2", [H1, H2], f32, kind="ExternalInput")
    bh2_d = nc.dram_tensor("bh2", [H2, 1], f32, kind="ExternalInput")
    wout_d = nc.dram_tensor("woutm", [H2, NCHUNK * NCHUNK], f32, kind="ExternalInput")
    ones_d = nc.dram_tensor("onesm", [D + 1, NCHUNK * NCHUNK], f32, kind="ExternalInput")
    iden_d = nc.dram_tensor("iden", [P, P], f16, kind="ExternalInput")
    out_d = nc.dram_tensor("out", [NCHUNK, P], f32, kind="ExternalOutput")

    with tile.TileContext(nc) as tc:
        with tc.tile_pool(name="const", bufs=1) as cp, \
             tc.tile_pool(name="emb", bufs=16) as ep, \
             tc.tile_pool(name="fold", bufs=6) as fp_, \
             tc.tile_pool(name="small", bufs=4) as sp, \
             tc.tile_pool(name="psumS", bufs=2, space="PSUM") as pps, \
             tc.tile_pool(name="psumM", bufs=1, space="PSUM") as ppm, \
             tc.tile_pool(name="psumF", bufs=1, space="PSUM") as ppf:
            idx_t = cp.tile([P, NCHUNK * F], i32)
            wh1_t = cp.tile([D + 1, H1], f32)
            wh2a_t = cp.tile([P, H2], f32)
            wh2b_t = cp.tile([P, H2], f32)
            bh2_t = cp.tile([H2, 1], f32)
            wout_t = cp.tile([H2, NCHUNK * NCHUNK], f32)
            ones_t = cp.tile([D + 1, NCHUNK * NCHUNK], f32)
            iden_t = cp.tile([P, P], f16)
            nc.sync.dma_start(idx_t[:], idx_d[:])
            nc.sync.dma_start(wh1_t[:], wh1_d[:])
            nc.sync.dma_start(wh2a_t[:], wh2_d[0:P, :])
            nc.sync.dma_start(wh2b_t[:], wh2_d[P : 2 * P, :])
            nc.sync.dma_start(bh2_t[:], bh2_d[:])
            nc.sync.dma_start(wout_t[:], wout_d[:])
            nc.sync.dma_start(ones_t[:], ones_d[:])
            nc.sync.dma_start(iden_t[:], iden_d[:])

            qn = 0
            for _r in range(repeats):
                fin_p = ppf.tile([NCHUNK, P], f32, tag="fin16", space="PSUM")
                for c in range(NCHUNK):
                    emb = ep.tile([P, F * E], f16, tag="emb")
                    for f in range(F):
                        _indirect_gather_q(
                            nc, emb[:, f * E : (f + 1) * E], w2_d[:],
                            idx_t[:, c * F + f : c * F + f + 1], qn % nq,
                        )
                        qn += 1

                    # Pairwise folds (s and ss together): 26 -> 13 -> 6 + carry
                    e13 = fp_.tile([P, 13 * E], f16, tag="e13")
                    nc.vector.tensor_tensor(
                        out=e13[:], in0=emb[:, : 13 * E], in1=emb[:, 13 * E :],
                        op=mybir.AluOpType.add,
                    )
                    e6 = fp_.tile([P, 6 * E], f16, tag="e6")
                    nc.vector.tensor_tensor(
                        out=e6[:], in0=e13[:, : 6 * E], in1=e13[:, 6 * E : 12 * E],
                        op=mybir.AluOpType.add,
                    )

                    # 7 transpose-matmuls accumulate [sT; ssT] = [128, P] PSUM
                    # (partitions 0:64 = sT, 64:128 = ssT).
                    sT_p = pps.tile([E, P], f32, tag="sT", space="PSUM")
                    for k in range(6):
                        nc.tensor.matmul(
                            out=sT_p[:], lhsT=e6[:, k * E : (k + 1) * E],
                            rhs=iden_t[:], start=(k == 0), stop=False,
                        )
                    nc.tensor.matmul(
                        out=sT_p[:], lhsT=e13[:, 12 * E : 13 * E], rhs=iden_t[:],
                        start=False, stop=True,
                    )

                    # fmTe [D+1, P]: rows 0:D = sT^2 - ssT, row D = 1.0.
                    fmTe = sp.tile([D + 1, P], f32, tag="fmTe")
                    nc.scalar.square(fmTe[0:D, :], sT_p[0:D, :])
                    nc.vector.memset(fmTe[D : D + 1, :], 1.0)
                    nc.vector.tensor_tensor(
                        out=fmTe[0:D, :], in0=fmTe[0:D, :], in1=sT_p[D : 2 * D, :],
                        op=mybir.AluOpType.subtract,
                    )

                    h1_p = ppm.tile([P, H1], f32, tag="h1", space="PSUM")
                    nc.tensor.matmul(
                        out=h1_p[:, 0:P], lhsT=wh1_t[:, 0:P], rhs=fmTe[:],
                        start=True, stop=True,
                    )
                    nc.tensor.matmul(
                        out=h1_p[:, P : 2 * P], lhsT=wh1_t[:, P : 2 * P],
                        rhs=fmTe[:], start=True, stop=True, skip_group_check=True,
                    )
                    h1 = sp.tile([P, H1], f32, tag="h1_sb")
                    nc.scalar.activation(
                        h1[:], h1_p[:], mybir.ActivationFunctionType.Sigmoid,
                        scale=0.005,
                    )

                    h2_p = ppm.tile([P, P], f32, tag="h2", space="PSUM")
                    nc.tensor.matmul(
                        out=h2_p[:], lhsT=wh2a_t[:], rhs=h1[:, 0:P],
                        start=True, stop=False,
                    )
                    nc.tensor.matmul(
                        out=h2_p[:], lhsT=wh2b_t[:], rhs=h1[:, P : 2 * P],
                        start=False, stop=True,
                    )
                    h2 = sp.tile([P, P], f32, tag="h2_sb")
                    nc.scalar.activation(
                        h2[:], h2_p[:], mybir.ActivationFunctionType.Sigmoid,
                        bias=bh2_t[:, 0:1],
                    )

                    # Chunk c deposits its output row into partition c of the
                    # shared fin16 PSUM tile via column-masked stationaries.
                    nc.tensor.matmul(
                        out=fin_p[:], lhsT=wout_t[:, c * NCHUNK : (c + 1) * NCHUNK],
                        rhs=h2[:], start=(c == 0), stop=False,
                    )
                    nc.tensor.matmul(
                        out=fin_p[:], lhsT=ones_t[:, c * NCHUNK : (c + 1) * NCHUNK],
                        rhs=fmTe[:], start=False, stop=(c == NCHUNK - 1),
                    )
                orow = sp.tile([NCHUNK, P], f32, tag="orow16")
                nc.vector.tensor_scalar_add(orow[:], fin_p[:], 0.0)
                nc.sync.dma_start(out_d[:], orow[:])
    nc.compile()
    return nc


def prep_inputs(cat_feat, W2, W_h1, b_h1, W_h2, b_h2, W_out, b_out, bias):
    cat = np.asarray(cat_feat).astype(np.int64)
    flat = (np.arange(F, dtype=np.int64)[None, :] * V + cat).astype(np.int32)
    idx = flat.reshape(NCORES, NCHUNK, P, F).transpose(0, 2, 1, 3).reshape(
        NCORES, P, NCHUNK * F
    )
    w2f = np.asarray(W2, dtype=np.float32).reshape(F * V, D)
    w2p = np.empty((F * V, E), dtype=np.float16)
    w2p[:, :D] = w2f.astype(np.float16)
    w2p[:, D:] = np.square(w2f).astype(np.float16)
    wh1e = np.concatenate(
        [
            np.asarray(W_h1, dtype=np.float32),
            (np.asarray(b_h1, dtype=np.float32) / np.float32(0.005))[None, :],
        ],
        axis=0,
    )
    wh2 = np.ascontiguousarray(np.asarray(W_h2, dtype=np.float32))
    bh2 = np.asarray(b_h2, dtype=np.float32).reshape(H2, 1)
    cb = np.float32(np.asarray(b_out).reshape(-1)[0]) + np.float32(
        np.asarray(bias).reshape(-1)[0]
    )
    # Column-masked stationaries: chunk c uses cols [16c,16c+16); col 16c+c
    # holds the real vector (ones-row carries cb), others zero.
    woutm = np.zeros((H2, NCHUNK * NCHUNK), dtype=np.float32)
    onesm = np.zeros((D + 1, NCHUNK * NCHUNK), dtype=np.float32)
    for c in range(NCHUNK):
        woutm[:, c * NCHUNK + c] = np.asarray(W_out, dtype=np.float32).reshape(-1)
        onesm[:D, c * NCHUNK + c] = 0.005
        onesm[D, c * NCHUNK + c] = cb
    iden = np.eye(P, dtype=np.float16)
    common = {
        "w2p": w2p, "wh1e": np.ascontiguousarray(wh1e), "wh2": wh2, "bh2": bh2,
        "woutm": woutm, "onesm": onesm, "iden": iden,
    }
    return [dict(common, idx=np.ascontiguousarray(idx[c])) for c in range(NCORES)]


def kernel(**inputs) -> np.ndarray:
    from concourse.bass_utils import run_bass_kernel_spmd

    if "nc" not in _cache:
        _cache["nc"] = build_program()
    nc = _cache["nc"]
    in_maps = prep_inputs(**inputs)
    res = run_bass_kernel_spmd(nc, in_maps, list(range(NCORES)))
    out = np.concatenate(
        [np.asarray(res.results[c]["out"]).reshape(BC) for c in range(NCORES)]
    )
    return out.astype(np.float32)[:, None]

